# revision 11
# baseline (speedup 1.0000x reference)
"""GQA attention block (RoPE + causal softmax + out-proj) on 8 TRN2 cores.

Sharding: 8 cores = 2 batches x 4 kv-pairs. Core c handles batch c//4 and
kv heads {2p, 2p+1} (p = c%4), i.e. q heads 6p..6p+5. Each core computes its
partial y^T = wo_slice^T @ attn_out^T; the host sums the 4 partials per batch
and transposes back.

Per-core layout: everything stays feature-major [d, s] so no on-device
transposes of large activations are needed:
  Q^T/K^T: [128d, s]   (projection emits them directly)
  scores come out transposed: [t, s] blocks from lhsT=K^T-slice, rhs=Q^T
  probs [t, s] feed AV directly with V in [t, dv] (via small PE transposes)
RoPE is applied in [d, s] form by permuting the head dim on the HOST to
[evens | odds]; the rotation becomes a partition-block swap (done with a PE
permutation matmul) plus elementwise mul/adds. The softmax scale is folded
into wq on the host. Softmax runs without max-subtraction (scores are O(10),
exp is safe in fp32).

v2 scheduling (PE-bound kernel, so everything serves PE occupancy):
 - qkv weights are resident in SBUF (loaded once, split across two queues);
   wo streams per-window on the gpsimd queue.
 - No per-i row-sum matmuls: probs are accumulated on the DVE into an fp32
   lacc; ONE ones-matmul per (unit, window) against the f32r view gives l,
   then the usual broadcast matmul + fast reciprocal.
 - Causal diagonal blocks are column-sliced: scores/exp/mask/AV/lacc only
   touch the valid [128r:512] range; the triangular boundary block gets a
   single [128,128] mask multiply.
 - The two units of a pair share one 2-bank PSUM score tile so exp / mask /
   lacc / final mul run as single batched ops over [128, 2, 512].
 - Projections for window j+1 are emitted as a chunk stream interleaved into
   attention/out-proj of window j, so the PE never drains while the ACT
   engine chews on exp.
"""

import math
from contextlib import ExitStack

import numpy as np
import ml_dtypes

import concourse.bass as bass
import concourse.mybir as mybir
import concourse.tile as tile
from concourse import bacc
from concourse.bass_utils import run_bass_kernel_spmd
from concourse.masks import make_identity

B, S, DIM = 2, 2048, 3072
NH, NKV, HD = 24, 8, 128
QT_PER_CORE = 6   # q head-tiles per core
KV_PER_CORE = 2   # kv heads per core
NDT = QT_PER_CORE + 2 * KV_PER_CORE  # 10 projection d-tiles
NKT = DIM // 128  # 24 contraction tiles
SW = 512          # s-window (matmul moving free dim)
NJ = S // SW      # 4 windows
NTT = S // 128    # 16 t-tiles
SCALE = 1.0 / math.sqrt(HD)

F32 = mybir.dt.float32
F32R = mybir.dt.float32r
BF16 = mybir.dt.bfloat16
BF = ml_dtypes.bfloat16

_PERM = np.concatenate([np.arange(0, HD, 2), np.arange(1, HD, 2)])

# dt order: pair0 (units 0,1 / kv 0) becomes runnable after dts {6,8,0,1}
DT_ORDER = [6, 8, 0, 1, 7, 9, 2, 3, 4, 5]
EXP = mybir.ActivationFunctionType.Exp


class Stream:
    def __init__(self, chunks=None):
        self.chunks = chunks or []
        self.pos = 0

    def emit(self, n):
        while n > 0 and self.pos < len(self.chunks):
            self.chunks[self.pos]()
            self.pos += 1
            n -= 1

    def finish(self):
        self.emit(len(self.chunks))


def _build_body(nc, tc, io, ctx):
    x4, w10, wo4 = io["x4"], io["w10"], io["wo4"]
    ropeC, ropeS, tri2, swp, yT = (
        io["ropeC"], io["ropeS"], io["tri2"], io["swp"], io["yT"])

    singles = ctx.enter_context(tc.tile_pool(name="singles", bufs=1))
    ps = ctx.enter_context(tc.tile_pool(name="ps", bufs=1, space=bass.MemorySpace.PSUM))
    xt_pool = ctx.enter_context(tc.tile_pool(name="xtp", bufs=2))
    wo_pool = ctx.enter_context(tc.tile_pool(name="wotp", bufs=6))
    raw_pool = ctx.enter_context(tc.tile_pool(name="rawp", bufs=3))
    vraw_pool = ctx.enter_context(tc.tile_pool(name="vrawp", bufs=2))
    qT_pool = ctx.enter_context(tc.tile_pool(name="qTp", bufs=13))
    probs_pool = ctx.enter_context(tc.tile_pool(name="prp", bufs=6))
    lacc_pool = ctx.enter_context(tc.tile_pool(name="laccp", bufs=2))
    laccr_pool = ctx.enter_context(tc.tile_pool(name="laccrp", bufs=1))
    small_pool = ctx.enter_context(tc.tile_pool(name="smp", bufs=1))
    out_pool = ctx.enter_context(tc.tile_pool(name="otp", bufs=4))
    y_pool = ctx.enter_context(tc.tile_pool(name="yp", bufs=2))

    # ---- constants / resident state ----
    ropeC_sb = singles.tile([128, S], BF16, tag="ropeC", name="ropeC_sb")
    ropeS_sb = singles.tile([128, S], BF16, tag="ropeS", name="ropeS_sb")
    tri2_sb = singles.tile([128, 2, 128], BF16, tag="tri2", name="tri2_sb")
    swp_sb = singles.tile([128, 128], BF16, tag="swp", name="swp_sb")
    ident = singles.tile([128, 128], F32, tag="ident", name="ident")
    ones_t = singles.tile([128, 1], BF16, tag="ones_t", name="ones_t")
    ones_mat = singles.tile([128, 128], F32R, tag="ones_mat", name="ones_mat")
    ones_mat0 = singles.tile([128, 128], F32, tag="ones_mat0", name="ones_mat0")
    w_all = singles.tile([128, NDT, NKT, 128], BF16, tag="w_all", name="w_all")

    # resident qkv weights: first tiles (in DT_ORDER) land first, split
    # across the sync and scalar queues with fine leading chunks
    nc.sync.dma_start(out=w_all[:, 6, 0:4], in_=w10[6, :, 0:4])
    nc.scalar.dma_start(out=w_all[:, 8, 0:4], in_=w10[8, :, 0:4])
    nc.sync.dma_start(out=w_all[:, 6, 4:12], in_=w10[6, :, 4:12])
    nc.scalar.dma_start(out=w_all[:, 8, 4:12], in_=w10[8, :, 4:12])
    nc.sync.dma_start(out=w_all[:, 6, 12:24], in_=w10[6, :, 12:24])
    nc.scalar.dma_start(out=w_all[:, 8, 12:24], in_=w10[8, :, 12:24])
    for n, dt in enumerate(DT_ORDER[2:]):
        eng = nc.sync if n % 2 == 0 else nc.scalar
        eng.dma_start(out=w_all[:, dt], in_=w10[dt])

    KT_sb = [singles.tile([128, S], BF16, tag=f"KT{g}", name=f"KT{g}")
             for g in range(KV_PER_CORE)]
    V_sb = [singles.tile([128, NTT, 128], BF16, tag=f"V{g}", name=f"V{g}")
            for g in range(KV_PER_CORE)]

    def load_consts():
        # after xt(0) on the gpsimd queue: rope tables are first needed a
        # full dt-chain (~7us) into window 0
        nc.gpsimd.dma_start(out=ropeC_sb, in_=ropeC[:])
        nc.gpsimd.dma_start(out=ropeS_sb, in_=ropeS[:])
        nc.gpsimd.dma_start(out=tri2_sb, in_=tri2[:])
        nc.gpsimd.dma_start(out=swp_sb, in_=swp[:])
        make_identity(nc, ident)
        nc.vector.memset(ones_t, 1.0)
        nc.vector.memset(ones_mat0, 1.0)
        nc.scalar.copy(out=ones_mat, in_=ones_mat0)

    def load_x(j):
        xt = xt_pool.tile([128, NKT, SW], BF16, name="xt")
        if j == 0:
            xsl = [(0, 1), (1, 2), (2, 4), (4, 6), (6, 9), (9, 12),
                   (12, 16), (16, 20), (20, 24)]
        else:
            xsl = [(0, 3), (3, 6), (6, 9), (9, 12), (12, 16), (16, 20),
                   (20, 24)]
        for k0, k1 in xsl:
            nc.gpsimd.dma_start(out=xt[:, k0:k1, :], in_=x4[j, :, k0:k1, :])
        return xt

    # ---- projections for a window, as a chunk stream ----
    def make_proj_stream(j, xt):
        jw = bass.ts(j, SW)
        qTj = [None] * QT_PER_CORE
        chunks = []
        post = []

        def _post_q(dt, raw, sw_ps):
            def run():
                if dt < 6:
                    qt = qT_pool.tile([128, SW], BF16, name="qt")
                    qTj[dt] = qt
                    dest = qt
                else:
                    dest = KT_sb[dt - 6][:, jw]
                nc.vector.tensor_mul(dest, raw, ropeC_sb[:, jw])
                t2 = raw_pool.tile([128, SW], BF16, tag="t2", bufs=2, name="t2")
                nc.vector.tensor_mul(t2, sw_ps, ropeS_sb[:, jw])
                nc.vector.tensor_add(dest, dest, t2)
            return run

        def _post_v(dt, vraw):
            def run():
                g = dt - 8
                tp = ps.tile([128, SW], F32, tag="pp", bufs=2, name="tp")
                for rr in range(4):
                    nc.tensor.transpose(tp[:, bass.ts(rr, 128)],
                                        vraw[:, bass.ts(rr, 128)], ident)
                nc.scalar.copy(out=V_sb[g][:, 4 * j:4 * j + 4, :],
                               in_=tp.rearrange("p (r t) -> p r t", r=4))
            return run

        for dt in DT_ORDER:
            kind = "q" if dt < 6 else ("k" if dt < 8 else "v")
            st = {}

            def _open(dt=dt, st=st):
                st["pp"] = ps.tile([128, SW], F32, tag="pp", bufs=2, name="pp")
                for k in range(6):
                    nc.tensor.matmul(st["pp"], w_all[:, dt, k, :], xt[:, k, :],
                                     start=(k == 0), stop=False,
                                     skip_group_check=True)
            chunks.append(_open)

            def _mid(dt=dt, st=st, k0=6, k1=12):
                for k in range(k0, k1):
                    nc.tensor.matmul(st["pp"], w_all[:, dt, k, :], xt[:, k, :],
                                     start=False, stop=False,
                                     skip_group_check=True)
            chunks.append(_mid)
            chunks.append(lambda dt=dt, st=st: _mid(dt, st, 12, 18))

            def _close(dt=dt, st=st, kind=kind):
                for k in range(18, NKT):
                    nc.tensor.matmul(st["pp"], w_all[:, dt, k, :], xt[:, k, :],
                                     start=False, stop=(k == NKT - 1),
                                     skip_group_check=True)
                pp = st["pp"]
                if kind == "v":
                    vraw = vraw_pool.tile([128, SW], F32, name="vraw")
                    nc.vector.tensor_copy(out=vraw, in_=pp)
                    post.append(_post_v(dt, vraw))
                else:
                    raw = raw_pool.tile([128, SW], BF16, tag="raw", bufs=3,
                                        name="raw")
                    nc.vector.tensor_copy(out=raw, in_=pp)
                    sw_ps = ps.tile([128, SW], F32, tag="pp", bufs=2,
                                    name="sw_ps")
                    nc.tensor.matmul(sw_ps, swp_sb, raw, start=True, stop=True,
                                     skip_group_check=True)
                    post.append(_post_q(dt, raw, sw_ps))
                if len(post) > 1:
                    post.pop(0)()
            chunks.append(_close)

        chunks.append(lambda: post.pop(0)())
        return Stream(chunks), qTj

    # ---- attention for a window ----
    def attn_window(j, qTj, hook):
        ni = 4 * j + 4
        nlast = ni - 1
        outTj = []
        for pair in range(3):
            u0, u1 = 2 * pair, 2 * pair + 1
            g0, g1 = u0 // 3, u1 // 3
            av2 = ps.tile([128, 2, SW], F32, tag="av", bufs=1, name="av2")
            lacc2 = lacc_pool.tile([128, 2, SW], F32, name="lacc2")
            pend = None
            for i in range(ni):
                r = i - 4 * j
                cr = 128 * r if r > 0 else 0
                cs = slice(cr, SW)
                sc2 = ps.tile([128, 2, SW], F32, tag="sc2", bufs=2, name="sc2")
                nc.tensor.matmul(sc2[:, 0, cs], KT_sb[g0][:, bass.ts(i, 128)],
                                 qTj[u0][:, cs], start=True, stop=True,
                                 skip_group_check=True)
                nc.tensor.matmul(sc2[:, 1, cs], KT_sb[g1][:, bass.ts(i, 128)],
                                 qTj[u1][:, cs], start=True, stop=True,
                                 skip_group_check=True)
                pr2 = probs_pool.tile([128, 2, SW], BF16, name="pr2")
                nc.scalar.activation(out=pr2[:, :, cs], in_=sc2[:, :, cs],
                                     func=EXP)
                if r >= 0:
                    # on DVE BEFORE any interleaved rope posts: the next AV
                    # matmul waits on this mask, not on lacc
                    nc.vector.tensor_mul(pr2[:, :, cr:cr + 128],
                                         pr2[:, :, cr:cr + 128], tri2_sb)
                if pend is not None:
                    pi, ppr2, pcs = pend
                    nc.tensor.matmul(av2[:, 0, pcs], V_sb[g0][:, pi, :],
                                     ppr2[:, 0, pcs], start=(pi == 0),
                                     stop=(pi == nlast), skip_group_check=True)
                    nc.tensor.matmul(av2[:, 1, pcs], V_sb[g1][:, pi, :],
                                     ppr2[:, 1, pcs], start=(pi == 0),
                                     stop=(pi == nlast), skip_group_check=True)
                pend = (i, pr2, cs)
                hook(1)
                if i == 0:
                    nc.vector.tensor_copy(out=lacc2, in_=pr2)
                else:
                    nc.vector.tensor_add(lacc2[:, :, cs], lacc2[:, :, cs],
                                         pr2[:, :, cs])
            pi, ppr2, pcs = pend
            nc.tensor.matmul(av2[:, 0, pcs], V_sb[g0][:, pi, :],
                             ppr2[:, 0, pcs], start=(pi == 0),
                             stop=(pi == nlast), skip_group_check=True)
            nc.tensor.matmul(av2[:, 1, pcs], V_sb[g1][:, pi, :],
                             ppr2[:, 1, pcs], start=(pi == 0),
                             stop=(pi == nlast), skip_group_check=True)

            # tail: l = colsum(lacc), broadcast, reciprocal, normalize
            la_b = laccr_pool.tile([128, 2, SW], BF16, name="la_b")
            nc.gpsimd.tensor_copy(out=la_b, in_=lacc2)
            lp = ps.tile([128, SW], F32, tag="sc2", bufs=2, name="lp")
            nc.tensor.matmul(lp[0:1, :], ones_t, la_b[:, 0, :], start=True,
                             stop=True, skip_group_check=True)
            nc.tensor.matmul(lp[32:33, :], ones_t, la_b[:, 1, :], start=True,
                             stop=True, skip_group_check=True)
            l_sbp = small_pool.tile([64, SW], F32R, tag="l_sbp", name="l_sbp")
            nc.scalar.copy(out=l_sbp[0:1, :], in_=lp[0:1, :])
            nc.scalar.copy(out=l_sbp[32:33, :], in_=lp[32:33, :])
            rb2 = ps.tile([128, 2, SW], F32, tag="sc2", bufs=2, name="rb2")
            nc.tensor.matmul(rb2[:, 0, :], ones_mat[0:1, :], l_sbp[0:1, :],
                             start=True, stop=True, skip_group_check=True)
            nc.tensor.matmul(rb2[:, 1, :], ones_mat[32:33, :], l_sbp[32:33, :],
                             start=True, stop=True, skip_group_check=True)
            rbs = small_pool.tile([128, 2, SW], F32, tag="rbs", name="rbs")
            nc.vector.reciprocal_approx_fast(out=rbs, in_=rb2)
            ot2 = out_pool.tile([128, 2, SW], BF16, name="ot2")
            nc.vector.tensor_mul(ot2, av2, rbs)
            outTj.append(ot2)
            hook(3)
        return outTj

    # ---- out-projection for a window ----
    def outproj_window(j, outTj, hook):
        jw = bass.ts(j, SW)
        for dd in range(NKT):
            wot = wo_pool.tile([128, QT_PER_CORE, 128], BF16, name="wot")
            nc.gpsimd.dma_start(out=wot, in_=wo4[dd])
            yp = ps.tile([128, SW], F32, tag="sc2", bufs=2, name="yp")
            for pair in range(3):
                for a in range(2):
                    u = 2 * pair + a
                    nc.tensor.matmul(yp, wot[:, u, :], outTj[pair][:, a, :],
                                     start=(u == 0), stop=(u == 5),
                                     skip_group_check=True)
            ys = y_pool.tile([128, SW], BF16, name="ys")
            nc.scalar.copy(out=ys, in_=yp)
            nc.sync.dma_start(out=yT[bass.ts(dd, 128), jw], in_=ys)
            hook(2)

    # ---- window pipeline ----
    xt0 = load_x(0)
    load_consts()
    s0, qT0 = make_proj_stream(0, xt0)
    s0.finish()
    qTjs = {0: qT0}
    for j in range(NJ):
        if j + 1 < NJ:
            xt_n = load_x(j + 1)
            nxt, qTjs[j + 1] = make_proj_stream(j + 1, xt_n)
        else:
            nxt = Stream()
        outTj = attn_window(j, qTjs[j], hook=nxt.emit)
        outproj_window(j, outTj, hook=nxt.emit)
        nxt.finish()


def build_nc():
    nc = bacc.Bacc("TRN2", target_bir_lowering=False, debug=False, num_devices=8)
    io = {
        "x4": nc.dram_tensor("x4", [NJ, 128, NKT, SW], BF16, kind="ExternalInput"),
        "w10": nc.dram_tensor("w10", [NDT, 128, NKT, 128], BF16, kind="ExternalInput"),
        "wo4": nc.dram_tensor("wo4", [NKT, 128, QT_PER_CORE, 128], BF16,
                              kind="ExternalInput"),
        "ropeC": nc.dram_tensor("ropeC", [HD, S], BF16, kind="ExternalInput"),
        "ropeS": nc.dram_tensor("ropeS", [HD, S], BF16, kind="ExternalInput"),
        "tri2": nc.dram_tensor("tri2", [128, 2, 128], BF16, kind="ExternalInput"),
        "swp": nc.dram_tensor("swp", [128, 128], BF16, kind="ExternalInput"),
        "yT": nc.dram_tensor("yT", [DIM, S], BF16, kind="ExternalOutput"),
    }
    with tile.TileContext(nc) as tc:
        with ExitStack() as ctx:
            _build_body(nc, tc, io, ctx)
    nc.compile()
    return nc


_NC = None


def _get_nc():
    global _NC
    if _NC is None:
        _NC = build_nc()
    return _NC


def make_in_maps(x, wq, wk, wv, wo, freqs_cos, freqs_sin):
    x = np.asarray(x, np.float32)
    wq = np.asarray(wq, np.float32)
    wk = np.asarray(wk, np.float32)
    wv = np.asarray(wv, np.float32)
    wo = np.asarray(wo, np.float32)
    cos = np.asarray(freqs_cos, np.float32)
    sin = np.asarray(freqs_sin, np.float32)

    wq_p = (wq.reshape(DIM, NH, HD)[:, :, _PERM] * SCALE).astype(BF)
    wk_p = wk.reshape(DIM, NKV, HD)[:, :, _PERM].astype(BF)
    wv_r = wv.reshape(DIM, NKV, HD).astype(BF)
    wo_r = wo.reshape(NH, HD, DIM)

    ropeC = np.ascontiguousarray(np.concatenate([cos.T, cos.T], 0)).astype(BF)
    ropeS = np.ascontiguousarray(np.concatenate([-sin.T, sin.T], 0)).astype(BF)

    tt = np.arange(128)[:, None]
    ss = np.arange(128)[None, :]
    tri = (tt <= ss).astype(BF)
    tri2 = np.ascontiguousarray(np.stack([tri, tri], axis=1))  # [128,2,128]

    swp = np.zeros((128, 128), BF)
    swp[np.arange(128), (np.arange(128) + 64) % 128] = 1.0

    in_maps = []
    for c in range(8):
        b, p = divmod(c, 4)
        wq_c = wq_p[:, 6 * p:6 * p + 6, :]          # [DIM, 6, 128]
        wk_c = wk_p[:, 2 * p:2 * p + 2, :]          # [DIM, 2, 128]
        wv_c = wv_r[:, 2 * p:2 * p + 2, :]          # [DIM, 2, 128]
        wcat = np.concatenate([wq_c, wk_c, wv_c], axis=1)   # [DIM, 10, 128]
        w10 = np.ascontiguousarray(
            wcat.reshape(NKT, 128, NDT, HD).transpose(2, 1, 0, 3))
        wo_c = wo_r[6 * p:6 * p + 6]                 # [6, 128, DIM]
        wo4 = np.ascontiguousarray(
            wo_c.reshape(QT_PER_CORE, HD, NKT, 128).transpose(2, 1, 0, 3)).astype(BF)
        xT_b = x[b].T                                 # [DIM, S]
        x4 = np.ascontiguousarray(
            xT_b.reshape(NKT, 128, NJ, SW).transpose(2, 1, 0, 3)).astype(BF)
        in_maps.append({
            "x4": x4,
            "w10": w10,
            "wo4": wo4,
            "ropeC": ropeC,
            "ropeS": ropeS,
            "tri2": tri2,
            "swp": swp,
        })
    return in_maps


def gather(results):
    y = np.empty((B, S, DIM), np.float32)
    for b in range(B):
        acc = results[4 * b]["yT"].astype(np.float32)
        for p in range(1, 4):
            acc = acc + results[4 * b + p]["yT"].astype(np.float32)
        y[b] = acc.T
    return y


def kernel(x, wq, wk, wv, wo, freqs_cos, freqs_sin, **run_kwargs):
    nc = _get_nc()
    in_maps = make_in_maps(x, wq, wk, wv, wo, freqs_cos, freqs_sin)
    res = run_bass_kernel_spmd(nc, in_maps, core_ids=list(range(8)), **run_kwargs)
    out = gather(res.results)
    if run_kwargs:
        return out, res
    return out


# revision 21
# speedup vs baseline: 1.1353x; 1.1353x over previous
"""GQA attention block (RoPE + causal softmax + out-proj) on 8 TRN2 cores.

Sharding: 8 cores = 2 batches x 4 kv-pairs. Core c handles batch c//4 and
kv heads {2p, 2p+1} (p = c%4), i.e. q heads 6p..6p+5. Each core computes its
partial y^T = wo_slice^T @ attn_out^T; the host sums the 4 partials per batch
and transposes back.

Per-core layout: everything stays feature-major [d, s] so no on-device
transposes of large activations are needed:
  Q^T/K^T: [128d, s]   (projection emits them directly)
  scores come out transposed: [t, s] blocks from lhsT=K^T-slice, rhs=Q^T
  probs [t, s] feed AV directly with V in [t, dv] (via small PE transposes)
RoPE is applied in [d, s] form by permuting the head dim on the HOST to
[evens | odds]; the rotation becomes a partition-block swap (done with a PE
permutation matmul) plus elementwise mul/adds. The softmax scale is folded
into wq on the host. Softmax runs without max-subtraction (scores are O(10),
exp is safe in fp32).

v2 scheduling (PE-bound kernel, so everything serves PE occupancy):
 - qkv weights are resident in SBUF (loaded once, split across two queues);
   wo streams per-window on the gpsimd queue.
 - No per-i row-sum matmuls: probs are accumulated on the DVE into an fp32
   lacc; ONE ones-matmul per (unit, window) against the f32r view gives l,
   then the usual broadcast matmul + fast reciprocal.
 - Causal diagonal blocks are column-sliced: scores/exp/mask/AV/lacc only
   touch the valid [128r:512] range; the triangular boundary block gets a
   single [128,128] mask multiply.
 - The two units of a pair share one 2-bank PSUM score tile so exp / mask /
   lacc / final mul run as single batched ops over [128, 2, 512].
 - Projections for window j+1 are emitted as a chunk stream interleaved into
   attention/out-proj of window j, so the PE never drains while the ACT
   engine chews on exp.
"""

import math
from contextlib import ExitStack

import numpy as np
import ml_dtypes

import concourse.bass as bass
import concourse.mybir as mybir
import concourse.tile as tile
from concourse import bacc
from concourse.bass_utils import run_bass_kernel_spmd
from concourse.masks import make_identity

B, S, DIM = 2, 2048, 3072
NH, NKV, HD = 24, 8, 128
QT_PER_CORE = 6   # q head-tiles per core
KV_PER_CORE = 2   # kv heads per core
NDT = QT_PER_CORE + 2 * KV_PER_CORE  # 10 projection d-tiles
NKT = DIM // 128  # 24 contraction tiles
SW = 512          # s-window (matmul moving free dim)
NJ = S // SW      # 4 windows
NTT = S // 128    # 16 t-tiles
SCALE = 1.0 / math.sqrt(HD)

F32 = mybir.dt.float32
F32R = mybir.dt.float32r
BF16 = mybir.dt.bfloat16
BF = ml_dtypes.bfloat16

_PERM = np.concatenate([np.arange(0, HD, 2), np.arange(1, HD, 2)])

# dt order: pair0 (units 0,1 / kv 0) becomes runnable after dts {6,8,0,1}
DT_ORDER = [6, 8, 0, 1, 7, 9, 2, 3, 4, 5]
EXP = mybir.ActivationFunctionType.Exp


class Stream:
    def __init__(self, chunks=None):
        self.chunks = chunks or []
        self.pos = 0

    def emit(self, n):
        while n > 0 and self.pos < len(self.chunks):
            self.chunks[self.pos]()
            self.pos += 1
            n -= 1

    def finish(self):
        self.emit(len(self.chunks))


class Paced:
    """Emit one chunk of `stream` per `stride` hook credits."""

    def __init__(self, stream, stride):
        self.stream = stream
        self.stride = stride
        self.ctr = 0

    def emit(self, n):
        self.ctr += n
        while self.ctr >= self.stride and self.stream.pos < len(self.stream.chunks):
            self.stream.emit(1)
            self.ctr -= self.stride


def _build_body(nc, tc, io, ctx):
    x4, w10, wo4 = io["x4"], io["w10"], io["wo4"]
    ropeC, ropeS, tri2, swp, yT = (
        io["ropeC"], io["ropeS"], io["tri2"], io["swp"], io["yT"])

    singles = ctx.enter_context(tc.tile_pool(name="singles", bufs=1))
    ps = ctx.enter_context(tc.tile_pool(name="ps", bufs=1, space=bass.MemorySpace.PSUM))
    xt_pool = ctx.enter_context(tc.tile_pool(name="xtp", bufs=2))
    wo_pool = ctx.enter_context(tc.tile_pool(name="wotp", bufs=4))
    raw_pool = ctx.enter_context(tc.tile_pool(name="rawp", bufs=3))
    vraw_pool = ctx.enter_context(tc.tile_pool(name="vrawp", bufs=2))
    qT_pool = ctx.enter_context(tc.tile_pool(name="qTp", bufs=13))
    probs_pool = ctx.enter_context(tc.tile_pool(name="prp", bufs=6))
    lacc_pool = ctx.enter_context(tc.tile_pool(name="laccp", bufs=2))
    laccr_pool = ctx.enter_context(tc.tile_pool(name="laccrp", bufs=1))
    small_pool = ctx.enter_context(tc.tile_pool(name="smp", bufs=1))
    out_pool = ctx.enter_context(tc.tile_pool(name="otp", bufs=6))
    y_pool = ctx.enter_context(tc.tile_pool(name="yp", bufs=2))

    # ---- constants / resident state ----
    ropeC_sb = singles.tile([128, S], BF16, tag="ropeC", name="ropeC_sb")
    ropeS_sb = singles.tile([128, S], BF16, tag="ropeS", name="ropeS_sb")
    tri2_sb = singles.tile([128, 2, 128], BF16, tag="tri2", name="tri2_sb")
    swp_sb = singles.tile([128, 128], BF16, tag="swp", name="swp_sb")
    ident = singles.tile([128, 128], F32, tag="ident", name="ident")
    ones_t = singles.tile([128, 1], BF16, tag="ones_t", name="ones_t")
    ones_mat = singles.tile([128, 128], F32R, tag="ones_mat", name="ones_mat")
    ones_mat0 = singles.tile([128, 128], F32, tag="ones_mat0", name="ones_mat0")
    w_all = singles.tile([128, NDT, NKT, 128], BF16, tag="w_all", name="w_all")

    KT_sb = [singles.tile([128, S], BF16, tag=f"KT{g}", name=f"KT{g}")
             for g in range(KV_PER_CORE)]
    V_sb = [singles.tile([128, NTT, 128], BF16, tag=f"V{g}", name=f"V{g}")
            for g in range(KV_PER_CORE)]

    def load_startup(xt):
        """x(0) + resident weights + consts, deadline-ordered over 3 queues.

        The first dt chain (dt6) consumes x[k] and w6[k] at ~0.27us/k, so x
        is split gpsimd/scalar, w6 leads on sync in fine chunks, and the
        rope tables load as per-window column slabs after the early x."""
        nc.gpsimd.dma_start(out=xt[:, 0:2, :], in_=x4[0, :, 0:2, :])
        nc.sync.dma_start(out=w_all[:, 6, 0:4], in_=w10[6, :, 0:4])
        nc.scalar.dma_start(out=xt[:, 12:15, :], in_=x4[0, :, 12:15, :])
        nc.gpsimd.dma_start(out=xt[:, 2:5, :], in_=x4[0, :, 2:5, :])
        nc.sync.dma_start(out=w_all[:, 6, 4:10], in_=w10[6, :, 4:10])
        nc.scalar.dma_start(out=xt[:, 15:18, :], in_=x4[0, :, 15:18, :])
        nc.gpsimd.dma_start(out=xt[:, 5:8, :], in_=x4[0, :, 5:8, :])
        nc.sync.dma_start(out=w_all[:, 6, 10:17], in_=w10[6, :, 10:17])
        nc.scalar.dma_start(out=xt[:, 18:21, :], in_=x4[0, :, 18:21, :])
        nc.gpsimd.dma_start(out=xt[:, 8:12, :], in_=x4[0, :, 8:12, :])
        nc.sync.dma_start(out=w_all[:, 6, 17:24], in_=w10[6, :, 17:24])
        nc.scalar.dma_start(out=xt[:, 21:24, :], in_=x4[0, :, 21:24, :])
        # chain 1 (dt8) weights + window-0 rope slab + small consts
        nc.sync.dma_start(out=w_all[:, 8, 0:12], in_=w10[8, :, 0:12])
        nc.scalar.dma_start(out=w_all[:, 8, 12:24], in_=w10[8, :, 12:24])
        nc.gpsimd.dma_start(out=ropeC_sb[:, 0:SW], in_=ropeC[:, 0:SW])
        nc.gpsimd.dma_start(out=ropeS_sb[:, 0:SW], in_=ropeS[:, 0:SW])
        nc.gpsimd.dma_start(out=tri2_sb, in_=tri2[:])
        nc.gpsimd.dma_start(out=swp_sb, in_=swp[:])
        # remaining dt tiles in halves, alternating queues; gpsimd takes the
        # trailing rope slabs
        for n, dt in enumerate(DT_ORDER[2:]):
            e1, e2 = (nc.sync, nc.scalar) if n % 2 == 0 else (nc.scalar, nc.sync)
            e1.dma_start(out=w_all[:, dt, 0:12], in_=w10[dt, :, 0:12])
            e2.dma_start(out=w_all[:, dt, 12:24], in_=w10[dt, :, 12:24])
        for j in range(1, NJ):
            jw = bass.ts(j, SW)
            nc.gpsimd.dma_start(out=ropeC_sb[:, jw], in_=ropeC[:, jw])
            nc.gpsimd.dma_start(out=ropeS_sb[:, jw], in_=ropeS[:, jw])
        make_identity(nc, ident)
        nc.vector.memset(ones_t, 1.0)
        nc.vector.memset(ones_mat0, 1.0)
        nc.scalar.copy(out=ones_mat, in_=ones_mat0)

    def load_x(j):
        xt = xt_pool.tile([128, NKT, SW], BF16, name="xt")
        xsl = [(0, 3), (3, 6), (6, 9), (9, 12), (12, 16), (16, 20), (20, 24)]
        for k0, k1 in xsl:
            nc.gpsimd.dma_start(out=xt[:, k0:k1, :], in_=x4[j, :, k0:k1, :])
        return xt

    # ---- projections for a window, as a chunk stream ----
    def make_proj_stream(j, xt):
        jw = bass.ts(j, SW)
        qTj = [None] * QT_PER_CORE
        chunks = []
        post = []

        def _post_q(dt, raw, sw_ps):
            def run():
                if dt < 6:
                    qt = qT_pool.tile([128, SW], BF16, name="qt")
                    qTj[dt] = qt
                    dest = qt
                else:
                    dest = KT_sb[dt - 6][:, jw]
                nc.vector.tensor_mul(dest, raw, ropeC_sb[:, jw])
                t2 = raw_pool.tile([128, SW], BF16, tag="t2", bufs=2, name="t2")
                nc.vector.tensor_mul(t2, sw_ps, ropeS_sb[:, jw])
                nc.vector.tensor_add(dest, dest, t2)
            return run

        def _post_v(dt, vraw):
            def run():
                g = dt - 8
                tp = ps.tile([128, SW], F32, tag="pp", bufs=2, name="tp")
                for rr in range(4):
                    nc.tensor.transpose(tp[:, bass.ts(rr, 128)],
                                        vraw[:, bass.ts(rr, 128)], ident)
                nc.scalar.copy(out=V_sb[g][:, 4 * j:4 * j + 4, :],
                               in_=tp.rearrange("p (r t) -> p r t", r=4))
            return run

        for dt in DT_ORDER:
            kind = "q" if dt < 6 else ("k" if dt < 8 else "v")
            st = {}

            def _open(dt=dt, st=st):
                st["pp"] = ps.tile([128, SW], F32, tag="pp", bufs=2, name="pp")
                for k in range(6):
                    nc.tensor.matmul(st["pp"], w_all[:, dt, k, :], xt[:, k, :],
                                     start=(k == 0), stop=False,
                                     skip_group_check=True)
            chunks.append(_open)

            def _mid(dt=dt, st=st, k0=6, k1=12):
                for k in range(k0, k1):
                    nc.tensor.matmul(st["pp"], w_all[:, dt, k, :], xt[:, k, :],
                                     start=False, stop=False,
                                     skip_group_check=True)
            chunks.append(_mid)
            chunks.append(lambda dt=dt, st=st: _mid(dt, st, 12, 18))

            def _close(dt=dt, st=st, kind=kind):
                for k in range(18, NKT):
                    nc.tensor.matmul(st["pp"], w_all[:, dt, k, :], xt[:, k, :],
                                     start=False, stop=(k == NKT - 1),
                                     skip_group_check=True)
                pp = st["pp"]
                if kind == "v":
                    vraw = vraw_pool.tile([128, SW], F32, name="vraw")
                    nc.vector.tensor_copy(out=vraw, in_=pp)
                    post.append(_post_v(dt, vraw))
                else:
                    raw = raw_pool.tile([128, SW], BF16, tag="raw", bufs=2,
                                        name="raw")
                    nc.vector.tensor_copy(out=raw, in_=pp)
                    sw_ps = ps.tile([128, SW], F32, tag="pp", bufs=2,
                                    name="sw_ps")
                    nc.tensor.matmul(sw_ps, swp_sb, raw, start=True, stop=True,
                                     skip_group_check=True)
                    post.append(_post_q(dt, raw, sw_ps))
                if len(post) > 1:
                    post.pop(0)()
            chunks.append(_close)

        chunks.append(lambda: post.pop(0)())
        return Stream(chunks), qTj

    # ---- attention for a window ----
    def attn_window(j, qTj, hook):
        ni = 4 * j + 4
        nlast = ni - 1
        outTj = []
        for pair in range(3):
            u0, u1 = 2 * pair, 2 * pair + 1
            g0, g1 = u0 // 3, u1 // 3
            av2 = ps.tile([128, 2, SW], F32, tag="av", bufs=1, name="av2")
            lacc2 = lacc_pool.tile([128, 2, SW], F32, name="lacc2")
            pend = None
            for i in range(ni):
                r = i - 4 * j
                cr = 128 * r if r > 0 else 0
                cs = slice(cr, SW)
                sc2 = ps.tile([128, 2, SW], F32, tag="sc2", bufs=2, name="sc2")
                nc.tensor.matmul(sc2[:, 0, cs], KT_sb[g0][:, bass.ts(i, 128)],
                                 qTj[u0][:, cs], start=True, stop=True,
                                 skip_group_check=True)
                nc.tensor.matmul(sc2[:, 1, cs], KT_sb[g1][:, bass.ts(i, 128)],
                                 qTj[u1][:, cs], start=True, stop=True,
                                 skip_group_check=True)
                pr2 = probs_pool.tile([128, 2, SW], BF16, name="pr2")
                nc.scalar.activation(out=pr2[:, :, cs], in_=sc2[:, :, cs],
                                     func=EXP)
                if r >= 0:
                    # on DVE BEFORE any interleaved rope posts: the next AV
                    # matmul waits on this mask, not on lacc
                    nc.vector.tensor_mul(pr2[:, :, cr:cr + 128],
                                         pr2[:, :, cr:cr + 128], tri2_sb)
                if pend is not None:
                    pi, ppr2, pcs = pend
                    nc.tensor.matmul(av2[:, 0, pcs], V_sb[g0][:, pi, :],
                                     ppr2[:, 0, pcs], start=(pi == 0),
                                     stop=(pi == nlast), skip_group_check=True)
                    nc.tensor.matmul(av2[:, 1, pcs], V_sb[g1][:, pi, :],
                                     ppr2[:, 1, pcs], start=(pi == 0),
                                     stop=(pi == nlast), skip_group_check=True)
                pend = (i, pr2, cs)
                hook(1)
                if i == 0:
                    nc.vector.tensor_copy(out=lacc2, in_=pr2)
                else:
                    nc.vector.tensor_add(lacc2[:, :, cs], lacc2[:, :, cs],
                                         pr2[:, :, cs])
            pi, ppr2, pcs = pend
            nc.tensor.matmul(av2[:, 0, pcs], V_sb[g0][:, pi, :],
                             ppr2[:, 0, pcs], start=(pi == 0),
                             stop=(pi == nlast), skip_group_check=True)
            nc.tensor.matmul(av2[:, 1, pcs], V_sb[g1][:, pi, :],
                             ppr2[:, 1, pcs], start=(pi == 0),
                             stop=(pi == nlast), skip_group_check=True)

            # tail: l = colsum(lacc), broadcast, reciprocal, normalize
            la_b = laccr_pool.tile([128, 2, SW], BF16, name="la_b")
            nc.scalar.copy(out=la_b, in_=lacc2)
            lp = ps.tile([128, SW], F32, tag="sc2", bufs=2, name="lp")
            nc.tensor.matmul(lp[0:1, :], ones_t, la_b[:, 0, :], start=True,
                             stop=True, skip_group_check=True)
            nc.tensor.matmul(lp[32:33, :], ones_t, la_b[:, 1, :], start=True,
                             stop=True, skip_group_check=True)
            l_sbp = small_pool.tile([64, SW], F32R, tag="l_sbp", name="l_sbp")
            nc.scalar.copy(out=l_sbp[0:1, :], in_=lp[0:1, :])
            nc.scalar.copy(out=l_sbp[32:33, :], in_=lp[32:33, :])
            rb2 = ps.tile([128, 2, SW], F32, tag="sc2", bufs=2, name="rb2")
            nc.tensor.matmul(rb2[:, 0, :], ones_mat[0:1, :], l_sbp[0:1, :],
                             start=True, stop=True, skip_group_check=True)
            nc.tensor.matmul(rb2[:, 1, :], ones_mat[32:33, :], l_sbp[32:33, :],
                             start=True, stop=True, skip_group_check=True)
            rbs = small_pool.tile([128, 2, SW], F32, tag="rbs", name="rbs")
            nc.vector.reciprocal_approx_fast(out=rbs, in_=rb2)
            ot2 = out_pool.tile([128, 2, SW], BF16, name="ot2")
            nc.vector.tensor_mul(ot2, av2, rbs)
            outTj.append(ot2)
            hook(3)
        return outTj

    # ---- out-projection for a window, as a chunk stream ----
    def make_outproj_stream(j, outTj):
        jw = bass.ts(j, SW)
        chunks = []
        for dd in range(NKT):
            def chunk(dd=dd):
                wot = wo_pool.tile([128, QT_PER_CORE, 128], BF16, name="wot")
                nc.gpsimd.dma_start(out=wot, in_=wo4[dd])
                yp = ps.tile([128, SW], F32, tag="sc2", bufs=2, name="yp")
                for pair in range(3):
                    for a in range(2):
                        u = 2 * pair + a
                        nc.tensor.matmul(yp, wot[:, u, :],
                                         outTj[pair][:, a, :],
                                         start=(u == 0), stop=(u == 5),
                                         skip_group_check=True)
                ys = y_pool.tile([128, SW], BF16, name="ys")
                nc.scalar.copy(out=ys, in_=yp)
                nc.sync.dma_start(out=yT[bass.ts(dd, 128), jw], in_=ys)
            chunks.append(chunk)
        return Stream(chunks)

    # ---- window pipeline ----
    # P(0) | A(0)+P(1) | O(0)+P(1) | A(1)+P(2) | O(1)+P(2) | A(2)+P(3) |
    # A(3)+O(2) | O(3): window 2's out-projection fills window 3's
    # (ACT-heavy, stream-less) attention.
    xt0 = xt_pool.tile([128, NKT, SW], BF16, name="xt")
    load_startup(xt0)
    s0, qT0 = make_proj_stream(0, xt0)
    s0.finish()
    qTjs = {0: qT0}
    outs = {}
    for j in (0, 1):
        xt_n = load_x(j + 1)
        nxt, qTjs[j + 1] = make_proj_stream(j + 1, xt_n)
        outs[j] = attn_window(j, qTjs[j], hook=nxt.emit)
        o = make_outproj_stream(j, outs[j])
        while o.pos < len(o.chunks):
            o.emit(1)
            nxt.emit(2)
        nxt.finish()
    xt3 = load_x(3)
    s3, qTjs[3] = make_proj_stream(3, xt3)
    outs[2] = attn_window(2, qTjs[2], hook=s3.emit)
    s3.finish()
    o2 = make_outproj_stream(2, outs[2])
    outs[3] = attn_window(3, qTjs[3], hook=Paced(o2, 2).emit)
    o2.finish()
    o3 = make_outproj_stream(3, outs[3])
    o3.finish()


def build_nc():
    nc = bacc.Bacc("TRN2", target_bir_lowering=False, debug=False, num_devices=8)
    io = {
        "x4": nc.dram_tensor("x4", [NJ, 128, NKT, SW], BF16, kind="ExternalInput"),
        "w10": nc.dram_tensor("w10", [NDT, 128, NKT, 128], BF16, kind="ExternalInput"),
        "wo4": nc.dram_tensor("wo4", [NKT, 128, QT_PER_CORE, 128], BF16,
                              kind="ExternalInput"),
        "ropeC": nc.dram_tensor("ropeC", [HD, S], BF16, kind="ExternalInput"),
        "ropeS": nc.dram_tensor("ropeS", [HD, S], BF16, kind="ExternalInput"),
        "tri2": nc.dram_tensor("tri2", [128, 2, 128], BF16, kind="ExternalInput"),
        "swp": nc.dram_tensor("swp", [128, 128], BF16, kind="ExternalInput"),
        "yT": nc.dram_tensor("yT", [DIM, S], BF16, kind="ExternalOutput"),
    }
    with tile.TileContext(nc) as tc:
        with ExitStack() as ctx:
            _build_body(nc, tc, io, ctx)
    nc.compile()
    return nc


_NC = None


def _get_nc():
    global _NC
    if _NC is None:
        _NC = build_nc()
    return _NC


def make_in_maps(x, wq, wk, wv, wo, freqs_cos, freqs_sin):
    x = np.asarray(x, np.float32)
    wq = np.asarray(wq, np.float32)
    wk = np.asarray(wk, np.float32)
    wv = np.asarray(wv, np.float32)
    wo = np.asarray(wo, np.float32)
    cos = np.asarray(freqs_cos, np.float32)
    sin = np.asarray(freqs_sin, np.float32)

    wq_p = (wq.reshape(DIM, NH, HD)[:, :, _PERM] * SCALE).astype(BF)
    wk_p = wk.reshape(DIM, NKV, HD)[:, :, _PERM].astype(BF)
    wv_r = wv.reshape(DIM, NKV, HD).astype(BF)
    wo_r = wo.reshape(NH, HD, DIM)

    ropeC = np.ascontiguousarray(np.concatenate([cos.T, cos.T], 0)).astype(BF)
    ropeS = np.ascontiguousarray(np.concatenate([-sin.T, sin.T], 0)).astype(BF)

    tt = np.arange(128)[:, None]
    ss = np.arange(128)[None, :]
    tri = (tt <= ss).astype(BF)
    tri2 = np.ascontiguousarray(np.stack([tri, tri], axis=1))  # [128,2,128]

    swp = np.zeros((128, 128), BF)
    swp[np.arange(128), (np.arange(128) + 64) % 128] = 1.0

    in_maps = []
    for c in range(8):
        b, p = divmod(c, 4)
        wq_c = wq_p[:, 6 * p:6 * p + 6, :]          # [DIM, 6, 128]
        wk_c = wk_p[:, 2 * p:2 * p + 2, :]          # [DIM, 2, 128]
        wv_c = wv_r[:, 2 * p:2 * p + 2, :]          # [DIM, 2, 128]
        wcat = np.concatenate([wq_c, wk_c, wv_c], axis=1)   # [DIM, 10, 128]
        w10 = np.ascontiguousarray(
            wcat.reshape(NKT, 128, NDT, HD).transpose(2, 1, 0, 3))
        wo_c = wo_r[6 * p:6 * p + 6]                 # [6, 128, DIM]
        wo4 = np.ascontiguousarray(
            wo_c.reshape(QT_PER_CORE, HD, NKT, 128).transpose(2, 1, 0, 3)).astype(BF)
        xT_b = x[b].T                                 # [DIM, S]
        x4 = np.ascontiguousarray(
            xT_b.reshape(NKT, 128, NJ, SW).transpose(2, 1, 0, 3)).astype(BF)
        in_maps.append({
            "x4": x4,
            "w10": w10,
            "wo4": wo4,
            "ropeC": ropeC,
            "ropeS": ropeS,
            "tri2": tri2,
            "swp": swp,
        })
    return in_maps


def gather(results):
    y = np.empty((B, S, DIM), np.float32)
    for b in range(B):
        acc = results[4 * b]["yT"].astype(np.float32)
        for p in range(1, 4):
            acc = acc + results[4 * b + p]["yT"].astype(np.float32)
        y[b] = acc.T
    return y


def kernel(x, wq, wk, wv, wo, freqs_cos, freqs_sin, **run_kwargs):
    nc = _get_nc()
    in_maps = make_in_maps(x, wq, wk, wv, wo, freqs_cos, freqs_sin)
    res = run_bass_kernel_spmd(nc, in_maps, core_ids=list(range(8)), **run_kwargs)
    out = gather(res.results)
    if run_kwargs:
        return out, res
    return out
